# revision 8
# baseline (speedup 1.0000x reference)
"""CRF (emission matmul + logsumexp-semiring scan + gold path) on 8 TRN2 cores.

Strategy (hardcoded for T=16384, D=2048, K=16, 8 cores):
  - Shard the time axis: core c owns timesteps [c*2048, (c+1)*2048).
  - Host pre-transposes/casts seq -> seqT bf16 [D, T] so each core streams its
    [2048, 2048] slab with perfectly contiguous DMAs and feeds the PE with the
    contraction dim (D) on partitions.
  - Emission on PE: out[k, t] accumulated over 16 d-chunks (lhsT = W.T chunk
    [128, 16] bf16, rhs = seqT tile [128, 512] bf16, PSUM f32), + b via DVE.
  - Partition function via a parallel semiring scan: each core's 2048 steps
    are split into 128 sub-chunks of L=16 steps. All 128 sub-chunk transfer
    matrices advance one step per PE matmul in *linear* space:
        state[(s,k), (b,j)] <- exp(emit_t - C) * sum_k' Texp[i,k'] state[k',j]
    with a single block-diagonal exp(transitions).T weight [128, 128] (8
    blocks of 16) and all 16 column-blocks (b) moving together. The per-step
    diagonal scale is one DVE tensor-tensor multiply against a per-step column
    of a rearranged exp(emit) table. 16 steps total per quarter-chain.
  - Cores ship raw emissions [16, 2048] and the 128 linear-space sub-chunk
    matrices [128, 256] back; the host combines 1024 16x16 matrices in f64
    (log-semiring) and computes the gold-path score from emissions.
"""

import numpy as np
import ml_dtypes

import concourse.bass as bass
import concourse.tile as tile
from concourse import bacc, mybir
from concourse.bass_utils import run_bass_kernel_spmd

BF16 = ml_dtypes.bfloat16

T, D, K = 16384, 2048, 16
NCORES = 8
TC = T // NCORES            # 2048 timesteps per core
L = 16                      # scan steps per sub-chunk
NB = 16                     # column blocks of the scan state (b)
NSL = 8                     # partition blocks of the scan state (s_l)
TBLK = 512                  # emission tile width (t)
NTB = TC // TBLK            # 4 quarters per core
NDCH = D // 128             # 16 contraction chunks
C_SHIFT = 3.3               # per-step log-space shift keeping f32 in range
QB = NB // NTB              # 4 column blocks per quarter


def _kernel_body(ctx, tc, seqT, wt, texp, bvec, init, emit_out, scan_out, reps=1):
    nc = tc.nc
    f32 = mybir.dt.float32
    bf16 = mybir.dt.bfloat16

    const_pool = ctx.enter_context(tc.tile_pool(name="const", bufs=1))
    seq_pool = ctx.enter_context(tc.tile_pool(name="seq", bufs=2 * NDCH))
    big_pool = ctx.enter_context(tc.tile_pool(name="big", bufs=1))
    state_pool = ctx.enter_context(tc.tile_pool(name="state", bufs=2))
    psum_e_pool = ctx.enter_context(tc.tile_pool(name="psum_e", bufs=2, space="PSUM"))
    psum_s_pool = ctx.enter_context(tc.tile_pool(name="psum_s", bufs=2, space="PSUM"))

    wt_t = const_pool.tile([128, NDCH * K], bf16)
    nc.sync.dma_start(out=wt_t[:], in_=wt)
    texp_t = const_pool.tile([128, 128], f32)
    nc.sync.dma_start(out=texp_t[:], in_=texp)
    bvec_t = const_pool.tile([K, 1], f32)
    nc.sync.dma_start(out=bvec_t[:], in_=bvec)
    cshift_t = const_pool.tile([K, 1], f32)
    nc.vector.memset(cshift_t[:], -C_SHIFT)

    for _rep in range(reps):
        _rep_body(
            nc, tc, seq_pool, big_pool, state_pool, psum_e_pool, psum_s_pool,
            wt_t, texp_t, bvec_t, cshift_t,
            seqT, init, emit_out, scan_out,
        )


def _rep_body(nc, tc, seq_pool, big_pool, state_pool, psum_e_pool, psum_s_pool,
              wt_t, texp_t, bvec_t, cshift_t, seqT, init, emit_out, scan_out):
    f32 = mybir.dt.float32
    bf16 = mybir.dt.bfloat16
    emit_sb = big_pool.tile([K, TC], f32, tag="emit_sb")
    exp_emit = big_pool.tile([K, TC], f32, tag="exp_emit")
    etab = big_pool.tile([128, NB, L], f32, tag="etab")
    # view of exp_emit with t decomposed as (b, s_l, tau)
    ee = exp_emit[:].rearrange("p (b sl tau) -> p b sl tau", b=NB, sl=NSL, tau=L)

    for q in range(NTB):
        tsl = bass.ts(q, TBLK)
        # ---- emission: psum[k, t] = sum_d W[k, d] * seq[t, d] ----
        seq_tiles = []
        for c in range(NDCH):
            st = seq_pool.tile([128, TBLK], bf16, tag="seqt")
            nc.sync.dma_start(
                out=st[:], in_=seqT[c * 128:(c + 1) * 128, tsl]
            )
            seq_tiles.append(st)
        pe = psum_e_pool.tile([K, TBLK], f32)
        for c in range(NDCH):
            nc.tensor.matmul(
                pe[:],
                wt_t[:, c * K:(c + 1) * K],
                seq_tiles[c][:],
                start=(c == 0),
                stop=(c == NDCH - 1),
            )
        nc.vector.tensor_scalar_add(emit_sb[:, tsl], pe[:], bvec_t[:, 0:1])
        nc.sync.dma_start(out=emit_out[:, tsl], in_=emit_sb[:, tsl])
        # exp(emit - C_SHIFT) on ACT
        nc.scalar.activation(
            out=exp_emit[:, tsl],
            in_=emit_sb[:, tsl],
            func=mybir.ActivationFunctionType.Exp,
            bias=cshift_t[:],
            scale=1.0,
        )
        # ---- rearrange into the per-step scale table ----
        # etab[(s_l, i), b, tau] = exp_emit[i, b*128 + s_l*16 + tau]
        for sl in range(NSL):
            nc.sync.dma_start(
                out=etab[sl * K:(sl + 1) * K, q * QB:(q + 1) * QB, :],
                in_=ee[:, q * QB:(q + 1) * QB, sl, :],
            )
        # ---- scan chain for this quarter's QB column blocks ----
        state = state_pool.tile([128, QB * K], f32, tag=f"st{q}")
        nc.sync.dma_start(
            out=state[:], in_=init[:, q * QB * K:(q + 1) * QB * K]
        )
        for tau in range(L):
            ps = psum_s_pool.tile([128, QB * K], f32)
            nc.tensor.matmul(ps[:], texp_t[:], state[:], start=True, stop=True)
            newst = state_pool.tile([128, QB * K], f32, tag=f"st{q}")
            nc.vector.tensor_mul(
                newst[:].rearrange("p (b j) -> p b j", b=QB),
                ps[:].rearrange("p (b j) -> p b j", b=QB),
                etab[:, q * QB:(q + 1) * QB, tau:tau + 1].broadcast_to(
                    [128, QB, K]
                ),
            )
            state = newst
        nc.sync.dma_start(
            out=scan_out[:, q * QB * K:(q + 1) * QB * K], in_=state[:]
        )


_PROGRAMS = {}


def _build_program(reps=1):
    if reps in _PROGRAMS:
        return _PROGRAMS[reps]
    from contextlib import ExitStack

    nc = bacc.Bacc(
        "TRN2", target_bir_lowering=False, debug=False, enable_asserts=False
    )
    f32 = mybir.dt.float32
    bf16 = mybir.dt.bfloat16
    seqT = nc.dram_tensor("seqt", [D, TC], bf16, kind="ExternalInput")
    wt = nc.dram_tensor("wt", [128, NDCH * K], bf16, kind="ExternalInput")
    texp = nc.dram_tensor("texp", [128, 128], f32, kind="ExternalInput")
    bvec = nc.dram_tensor("bvec", [K, 1], f32, kind="ExternalInput")
    init = nc.dram_tensor("init", [128, NB * K], f32, kind="ExternalInput")
    emit_out = nc.dram_tensor("emit_out", [K, TC], f32, kind="ExternalOutput")
    scan_out = nc.dram_tensor("scan_out", [128, NB * K], f32, kind="ExternalOutput")

    with tile.TileContext(nc) as tc:
        with ExitStack() as ctx:
            _kernel_body(
                ctx, tc,
                seqT.ap(), wt.ap(), texp.ap(), bvec.ap(), init.ap(),
                emit_out.ap(), scan_out.ap(), reps=reps,
            )
    nc.compile()
    _PROGRAMS[reps] = nc
    return nc


def _host_inputs(seq, W, b, transitions):
    """Build the per-core input maps (host-side preprocessing)."""
    seqT = np.ascontiguousarray(seq.T).astype(BF16)          # [D, T]
    # wt[p, c*16+i] = W[i, c*128+p]
    wt = np.ascontiguousarray(
        W.reshape(K, NDCH, 128).transpose(2, 1, 0).reshape(128, NDCH * K)
    ).astype(BF16)
    Texp = np.exp(transitions.astype(np.float64)).astype(np.float32)
    texp_bd = np.zeros((128, 128), dtype=np.float32)
    for s in range(NSL):
        texp_bd[s * K:(s + 1) * K, s * K:(s + 1) * K] = Texp.T
    bvec = np.ascontiguousarray(b.reshape(K, 1)).astype(np.float32)
    init = np.tile(np.eye(K, dtype=np.float32), (NSL, NB))   # [128, 256]
    in_maps = []
    for c in range(NCORES):
        in_maps.append({
            "seqt": np.ascontiguousarray(seqT[:, c * TC:(c + 1) * TC]),
            "wt": wt,
            "texp": texp_bd,
            "bvec": bvec,
            "init": np.ascontiguousarray(init),
        })
    return in_maps


def _lse(x, axis=None):
    m = np.max(x, axis=axis, keepdims=True)
    out = m + np.log(np.sum(np.exp(x - m), axis=axis, keepdims=True))
    return np.squeeze(out, axis=axis) if axis is not None else out.reshape(())


def _host_combine(emit, scan_mats, tags, trans_start, transitions, trans_end):
    """emit: [T, K] f32; scan_mats: [S, K, K] linear-space f32 (shift C_SHIFT/step)."""
    emit64 = emit.astype(np.float64)
    tr64 = transitions.astype(np.float64)
    tags = np.asarray(tags).astype(np.int64)

    alpha = trans_start.astype(np.float64) + emit64[0]
    for t in range(1, L):
        alpha = _lse(tr64 + alpha[None, :], axis=1) + emit64[t]
    logM = np.log(np.maximum(scan_mats.astype(np.float64), 1e-300)) + L * C_SHIFT
    S = logM.shape[0]
    for g in range(1, S):
        alpha = _lse(logM[g] + alpha[None, :], axis=1)
    log_z = _lse(trans_end.astype(np.float64) + alpha)

    gold = (
        trans_start.astype(np.float64)[tags[0]]
        + emit64[0, tags[0]]
        + tr64[tags[1:], tags[:-1]].sum()
        + emit64[np.arange(1, T), tags[1:]].sum()
        + trans_end.astype(np.float64)[tags[-1]]
    )
    return np.float32(gold - log_z)


def _run_device(in_maps, reps=1, **kwargs):
    nc = _build_program(reps)
    return run_bass_kernel_spmd(nc, in_maps, list(range(NCORES)), **kwargs)


def _decode_outputs(results):
    emit_parts = [np.asarray(results[c]["emit_out"]) for c in range(NCORES)]
    emit = np.concatenate(emit_parts, axis=1).T.astype(np.float32)  # [T, K]
    mats = []
    for c in range(NCORES):
        so = np.asarray(results[c]["scan_out"]).astype(np.float32)  # [128, 256]
        # rows (s_l, k), cols (b, j) -> sub-chunk g_local = b*NSL + s_l
        m = so.reshape(NSL, K, NB, K).transpose(2, 0, 1, 3).reshape(-1, K, K)
        mats.append(m)
    return emit, np.concatenate(mats, axis=0)  # [T,K], [1024, K, K]


def kernel(**inputs):
    seq = np.asarray(inputs["seq"], dtype=np.float32)
    tags = np.asarray(inputs["tags"])
    W = np.asarray(inputs["W"], dtype=np.float32)
    b = np.asarray(inputs["b"], dtype=np.float32)
    trans_start = np.asarray(inputs["trans_start"], dtype=np.float32)
    transitions = np.asarray(inputs["transitions"], dtype=np.float32)
    trans_end = np.asarray(inputs["trans_end"], dtype=np.float32)

    in_maps = _host_inputs(seq, W, b, transitions)
    results = _run_device(in_maps).results
    emit, scan_mats = _decode_outputs(results)
    return np.asarray(
        _host_combine(emit, scan_mats, tags, trans_start, transitions, trans_end)
    )


# revision 9
# speedup vs baseline: 285.2744x; 285.2744x over previous
"""CRF (emission matmul + logsumexp-semiring scan + gold path) on 8 TRN2 cores.

Strategy (hardcoded for T=16384, D=2048, K=16, 8 cores):
  - Shard the time axis: core c owns timesteps [c*2048, (c+1)*2048).
  - Host pre-transposes/casts seq -> seqT bf16 [D, T] so each core streams its
    [2048, 2048] slab with perfectly contiguous DMAs and feeds the PE with the
    contraction dim (D) on partitions.
  - Emission on PE: out[k, t] accumulated over 16 d-chunks (lhsT = W.T chunk
    [128, 16] bf16, rhs = seqT tile [128, 512] bf16, PSUM f32), + b via DVE.
  - Partition function via a parallel semiring scan: each core's 2048 steps
    are split into 128 sub-chunks of L=16 steps. All 128 sub-chunk transfer
    matrices advance one step per PE matmul in *linear* space:
        state[(s,k), (b,j)] <- exp(emit_t - C) * sum_k' Texp[i,k'] state[k',j]
    with a single block-diagonal exp(transitions).T weight [128, 128] (8
    blocks of 16) and all 16 column-blocks (b) moving together. The per-step
    diagonal scale is one DVE tensor-tensor multiply against a per-step column
    of a rearranged exp(emit) table. 16 steps total per quarter-chain.
  - Cores ship raw emissions [16, 2048] and the 128 linear-space sub-chunk
    matrices [128, 256] back; the host combines 1024 16x16 matrices in f64
    (log-semiring) and computes the gold-path score from emissions.
"""

import numpy as np
import ml_dtypes

import concourse.bass as bass
import concourse.tile as tile
from concourse import bacc, mybir
from concourse.bass_utils import run_bass_kernel_spmd

BF16 = ml_dtypes.bfloat16

T, D, K = 16384, 2048, 16
NCORES = 8
TC = T // NCORES            # 2048 timesteps per core
L = 16                      # scan steps per sub-chunk
NB = 16                     # column blocks of the scan state (b)
NSL = 8                     # partition blocks of the scan state (s_l)
TBLK = 512                  # emission tile width (t)
NTB = TC // TBLK            # 4 quarters per core
NDCH = D // 128             # 16 contraction chunks
C_SHIFT = 3.3               # per-step log-space shift keeping f32 in range
QB = NB // NTB              # 4 column blocks per quarter


def _kernel_body(ctx, tc, seqT, wt, texp, bvec, init, emit_out, scan_out, reps=1):
    nc = tc.nc
    f32 = mybir.dt.float32
    bf16 = mybir.dt.bfloat16

    const_pool = ctx.enter_context(tc.tile_pool(name="const", bufs=1))
    seq_pool = ctx.enter_context(tc.tile_pool(name="seq", bufs=2 * NDCH))
    big_pool = ctx.enter_context(tc.tile_pool(name="big", bufs=1))
    state_pool = ctx.enter_context(tc.tile_pool(name="state", bufs=2))
    psum_e_pool = ctx.enter_context(tc.tile_pool(name="psum_e", bufs=2, space="PSUM"))
    psum_s_pool = ctx.enter_context(tc.tile_pool(name="psum_s", bufs=2, space="PSUM"))

    wt_t = const_pool.tile([128, NDCH * K], bf16)
    nc.sync.dma_start(out=wt_t[:], in_=wt)
    texp_t = const_pool.tile([128, 128], f32)
    nc.sync.dma_start(out=texp_t[:], in_=texp)
    bvec_t = const_pool.tile([K, 1], f32)
    nc.sync.dma_start(out=bvec_t[:], in_=bvec)
    cshift_t = const_pool.tile([K, 1], f32)
    nc.vector.memset(cshift_t[:], -C_SHIFT)

    if isinstance(reps, tuple):  # hardware loop for differential timing
        n_loop = reps[0]
        with tc.For_i(0, n_loop, 1, hint_engines=(
            mybir.EngineType.SP, mybir.EngineType.PE, mybir.EngineType.DVE,
        )):
            _rep_body(
                nc, tc, seq_pool, big_pool, state_pool, psum_e_pool,
                psum_s_pool, wt_t, texp_t, bvec_t, cshift_t,
                seqT, init, emit_out, scan_out,
            )
        return
    for _rep in range(reps):
        _rep_body(
            nc, tc, seq_pool, big_pool, state_pool, psum_e_pool, psum_s_pool,
            wt_t, texp_t, bvec_t, cshift_t,
            seqT, init, emit_out, scan_out,
        )


def _rep_body(nc, tc, seq_pool, big_pool, state_pool, psum_e_pool, psum_s_pool,
              wt_t, texp_t, bvec_t, cshift_t, seqT, init, emit_out, scan_out):
    f32 = mybir.dt.float32
    bf16 = mybir.dt.bfloat16
    emit_sb = big_pool.tile([K, TC], f32, tag="emit_sb")
    exp_emit = big_pool.tile([K, TC], f32, tag="exp_emit")
    etab = big_pool.tile([128, NB, L], f32, tag="etab")
    # view of exp_emit with t decomposed as (b, s_l, tau)
    ee = exp_emit[:].rearrange("p (b sl tau) -> p b sl tau", b=NB, sl=NSL, tau=L)

    for q in range(NTB):
        tsl = bass.ts(q, TBLK)
        # ---- emission: psum[k, t] = sum_d W[k, d] * seq[t, d] ----
        seq_tiles = []
        for c in range(NDCH):
            st = seq_pool.tile([128, TBLK], bf16, tag="seqt")
            nc.sync.dma_start(
                out=st[:], in_=seqT[c * 128:(c + 1) * 128, tsl]
            )
            seq_tiles.append(st)
        pe = psum_e_pool.tile([K, TBLK], f32)
        for c in range(NDCH):
            nc.tensor.matmul(
                pe[:],
                wt_t[:, c * K:(c + 1) * K],
                seq_tiles[c][:],
                start=(c == 0),
                stop=(c == NDCH - 1),
            )
        nc.vector.tensor_scalar_add(emit_sb[:, tsl], pe[:], bvec_t[:, 0:1])
        nc.sync.dma_start(out=emit_out[:, tsl], in_=emit_sb[:, tsl])
        # exp(emit - C_SHIFT) on ACT
        nc.scalar.activation(
            out=exp_emit[:, tsl],
            in_=emit_sb[:, tsl],
            func=mybir.ActivationFunctionType.Exp,
            bias=cshift_t[:],
            scale=1.0,
        )
        # ---- rearrange into the per-step scale table ----
        # etab[(s_l, i), b, tau] = exp_emit[i, b*128 + s_l*16 + tau]
        for sl in range(NSL):
            nc.sync.dma_start(
                out=etab[sl * K:(sl + 1) * K, q * QB:(q + 1) * QB, :],
                in_=ee[:, q * QB:(q + 1) * QB, sl, :],
            )
        # ---- scan chain for this quarter's QB column blocks ----
        state = state_pool.tile([128, QB * K], f32, tag=f"st{q}")
        nc.sync.dma_start(
            out=state[:], in_=init[:, q * QB * K:(q + 1) * QB * K]
        )
        for tau in range(L):
            ps = psum_s_pool.tile([128, QB * K], f32)
            nc.tensor.matmul(ps[:], texp_t[:], state[:], start=True, stop=True)
            newst = state_pool.tile([128, QB * K], f32, tag=f"st{q}")
            nc.vector.tensor_mul(
                newst[:].rearrange("p (b j) -> p b j", b=QB),
                ps[:].rearrange("p (b j) -> p b j", b=QB),
                etab[:, q * QB:(q + 1) * QB, tau:tau + 1].broadcast_to(
                    [128, QB, K]
                ),
            )
            state = newst
        nc.sync.dma_start(
            out=scan_out[:, q * QB * K:(q + 1) * QB * K], in_=state[:]
        )


_PROGRAMS = {}


def _build_program(reps=1):
    if reps in _PROGRAMS:
        return _PROGRAMS[reps]
    from contextlib import ExitStack

    nc = bacc.Bacc(
        "TRN2", target_bir_lowering=False, debug=False, enable_asserts=False
    )
    f32 = mybir.dt.float32
    bf16 = mybir.dt.bfloat16
    seqT = nc.dram_tensor("seqt", [D, TC], bf16, kind="ExternalInput")
    wt = nc.dram_tensor("wt", [128, NDCH * K], bf16, kind="ExternalInput")
    texp = nc.dram_tensor("texp", [128, 128], f32, kind="ExternalInput")
    bvec = nc.dram_tensor("bvec", [K, 1], f32, kind="ExternalInput")
    init = nc.dram_tensor("init", [128, NB * K], f32, kind="ExternalInput")
    emit_out = nc.dram_tensor("emit_out", [K, TC], f32, kind="ExternalOutput")
    scan_out = nc.dram_tensor("scan_out", [128, NB * K], f32, kind="ExternalOutput")

    with tile.TileContext(nc) as tc:
        with ExitStack() as ctx:
            _kernel_body(
                ctx, tc,
                seqT.ap(), wt.ap(), texp.ap(), bvec.ap(), init.ap(),
                emit_out.ap(), scan_out.ap(), reps=reps,
            )
    nc.compile()
    _PROGRAMS[reps] = nc
    return nc


def _host_inputs(seq, W, b, transitions):
    """Build the per-core input maps (host-side preprocessing)."""
    seqT = np.ascontiguousarray(seq.T).astype(BF16)          # [D, T]
    # wt[p, c*16+i] = W[i, c*128+p]
    wt = np.ascontiguousarray(
        W.reshape(K, NDCH, 128).transpose(2, 1, 0).reshape(128, NDCH * K)
    ).astype(BF16)
    Texp = np.exp(transitions.astype(np.float64)).astype(np.float32)
    texp_bd = np.zeros((128, 128), dtype=np.float32)
    for s in range(NSL):
        texp_bd[s * K:(s + 1) * K, s * K:(s + 1) * K] = Texp.T
    bvec = np.ascontiguousarray(b.reshape(K, 1)).astype(np.float32)
    init = np.tile(np.eye(K, dtype=np.float32), (NSL, NB))   # [128, 256]
    in_maps = []
    for c in range(NCORES):
        in_maps.append({
            "seqt": np.ascontiguousarray(seqT[:, c * TC:(c + 1) * TC]),
            "wt": wt,
            "texp": texp_bd,
            "bvec": bvec,
            "init": np.ascontiguousarray(init),
        })
    return in_maps


def _lse(x, axis=None):
    m = np.max(x, axis=axis, keepdims=True)
    out = m + np.log(np.sum(np.exp(x - m), axis=axis, keepdims=True))
    return np.squeeze(out, axis=axis) if axis is not None else out.reshape(())


def _host_combine(emit, scan_mats, tags, trans_start, transitions, trans_end):
    """emit: [T, K] f32; scan_mats: [S, K, K] linear-space f32 (shift C_SHIFT/step)."""
    emit64 = emit.astype(np.float64)
    tr64 = transitions.astype(np.float64)
    tags = np.asarray(tags).astype(np.int64)

    alpha = trans_start.astype(np.float64) + emit64[0]
    for t in range(1, L):
        alpha = _lse(tr64 + alpha[None, :], axis=1) + emit64[t]
    logM = np.log(np.maximum(scan_mats.astype(np.float64), 1e-300)) + L * C_SHIFT
    S = logM.shape[0]
    for g in range(1, S):
        alpha = _lse(logM[g] + alpha[None, :], axis=1)
    log_z = _lse(trans_end.astype(np.float64) + alpha)

    gold = (
        trans_start.astype(np.float64)[tags[0]]
        + emit64[0, tags[0]]
        + tr64[tags[1:], tags[:-1]].sum()
        + emit64[np.arange(1, T), tags[1:]].sum()
        + trans_end.astype(np.float64)[tags[-1]]
    )
    return np.float32(gold - log_z)


def _run_device(in_maps, reps=1, **kwargs):
    nc = _build_program(reps)
    return run_bass_kernel_spmd(nc, in_maps, list(range(NCORES)), **kwargs)


def _decode_outputs(results):
    emit_parts = [np.asarray(results[c]["emit_out"]) for c in range(NCORES)]
    emit = np.concatenate(emit_parts, axis=1).T.astype(np.float32)  # [T, K]
    mats = []
    for c in range(NCORES):
        so = np.asarray(results[c]["scan_out"]).astype(np.float32)  # [128, 256]
        # rows (s_l, k), cols (b, j) -> sub-chunk g_local = b*NSL + s_l
        m = so.reshape(NSL, K, NB, K).transpose(2, 0, 1, 3).reshape(-1, K, K)
        mats.append(m)
    return emit, np.concatenate(mats, axis=0)  # [T,K], [1024, K, K]


def kernel(**inputs):
    seq = np.asarray(inputs["seq"], dtype=np.float32)
    tags = np.asarray(inputs["tags"])
    W = np.asarray(inputs["W"], dtype=np.float32)
    b = np.asarray(inputs["b"], dtype=np.float32)
    trans_start = np.asarray(inputs["trans_start"], dtype=np.float32)
    transitions = np.asarray(inputs["transitions"], dtype=np.float32)
    trans_end = np.asarray(inputs["trans_end"], dtype=np.float32)

    in_maps = _host_inputs(seq, W, b, transitions)
    results = _run_device(in_maps).results
    emit, scan_mats = _decode_outputs(results)
    return np.asarray(
        _host_combine(emit, scan_mats, tags, trans_start, transitions, trans_end)
    )
